# revision 1
# baseline (speedup 1.0000x reference)
"""CRF partition function (neg log partition) on 8 Trainium2 NeuronCores.

Algorithm: rank-1 chunked scan ("warmup stitch"), data-parallel over batch
(8 batches/core) and chunk-parallel over time. In prob space the recurrence is
p_t = p_{t-1} @ (E D_t), E = exp(log_transition) (row-stochastic),
D_t = diag(exp(obs_t - beta)). Products of positive matrices contract to
rank-1, so the T=4096 serial scan splits into C=256 independent chunks of
L=16 steps run as parallel matmul columns; each chunk starts from a uniform
probe with W=1 warmup step that recovers the true state *direction*;
per-chunk scale corrections are ratios of column sums, stitched on host in
fp64 (validated rel err ~1.2e-4 incl. bf16 quantization).

Device structure, tuned against the TRN2 engine cost model:
  * G = L+W = 17 serial steps over N = 2048 chains/core.
  * 4 independent column groups of 512 chains, each its own serial recurrence
    and its own PSUM bank. Group 0: DVE tensor_mul directly from f32 PSUM
    (1x rate). Groups 1-3: ScalarE evacuates PSUM -> bf16 SBUF (1x @1.2GHz),
    DVE multiplies bf16 at 2x_1P. Per step: ACT 3x~0.6us and DVE
    0.66+3x0.33us run concurrently; ACT is the critical engine.
  * Minimal HBM traffic: es stores only the L body slices (T*BPC*S bf16 =
    8.39MB/core, no warmup duplication). Warmup slices of chunk c+1 equal
    body slices g-L of chunk c shifted by BPC columns, so steps g>=L reuse
    the first W slices (persistent tiles) at a +BPC column offset; the last
    chunk's tail observations come from a tiny etail input.
  * Chunk-0 exact init at g=W-1 from SBUF (es slice W-1 cols 0:BPC ==
    exp(obs[:,0,:]-beta)) via DVE copy; ACT Copy table preloaded behind the
    initial DMAs; per-group output stores overlap the final steps.
"""

import numpy as np
import ml_dtypes

import concourse.bacc as bacc
import concourse.mybir as mybir
from concourse.tile import TileContext
from concourse.bass_utils import run_bass_kernel_spmd

bf16 = ml_dtypes.bfloat16

B, T, S = 64, 4096, 128
BETA = 0.5
NCORES = 8
BPC = B // NCORES      # 8 batches per core

L, W = 16, 1           # chunk length, warmup steps
C = T // L             # 256 chunks
G = L + W              # 19 serial steps
N = C * BPC            # 2048 chains per core

GW = 512               # column-group width (one PSUM bank of f32)
NG = N // GW           # 4 groups


def configure(l, w):
    global L, W, C, G, N, NG
    L, W = l, w
    C = T // L
    G = L + W
    N = C * BPC
    NG = N // GW


def _build_device_program(esbufs=6, xbufs=2, ndirect=1, repeats=1, dma_only=0, no_dma=0, gw=0, psbufs=1, dlast=0, park=1, hoist=0, esplit=0, evbufs=3, wspread=1, esfirst=0, psplit=1):
    """ndirect: how many of the NG groups use the direct-DVE path (rest ACT).
    dma_only: emit only the es DMAs (bandwidth microbench).
    no_dma: all steps read one resident es tile (compute-pipeline microbench;
    output is numerically wrong).
    gw: column-group width override (default GW)."""
    gw = gw or GW
    ng = N // gw
    nc = bacc.Bacc("TRN2", target_bir_lowering=False)
    es_d = nc.dram_tensor("eslices", [L, S, N], mybir.dt.bfloat16, kind="ExternalInput")
    et_d = nc.dram_tensor("etail", [S, W * BPC], mybir.dt.bfloat16, kind="ExternalInput")
    ew_d = nc.dram_tensor("ew", [S, S], mybir.dt.bfloat16, kind="ExternalInput")
    wout_d = nc.dram_tensor("wout", [S, N], mybir.dt.bfloat16, kind="ExternalOutput")
    yout_d = nc.dram_tensor("yout", [S, N], mybir.dt.bfloat16, kind="ExternalOutput")

    with TileContext(nc) as tc:
        with (
            tc.tile_pool(name="const", bufs=1) as cpool,
            tc.tile_pool(name="pers", bufs=W) as perspool,
            tc.tile_pool(name="es", bufs=esbufs) as espool,
            tc.tile_pool(name="state", bufs=xbufs) as xpool,
            tc.tile_pool(name="ev", bufs=evbufs) as evpool,
            tc.tile_pool(name="ps", bufs=psbufs, space="PSUM") as ppool,
        ):
            # preload the ACT Copy table set while the first DMAs stream in
            warm = cpool.tile([1, 2], mybir.dt.bfloat16)
            nc.scalar.copy(out=warm[0:1, 0:1], in_=warm[0:1, 1:2])

            pers0 = None
            if esfirst and not (dma_only or no_dma):
                pers0 = perspool.tile([S, N], mybir.dt.bfloat16, tag="pers")
                nc.sync.dma_start(out=pers0[:], in_=es_d[0])
            E_sb = cpool.tile([S, S], mybir.dt.bfloat16)
            nc.sync.dma_start(out=E_sb[:], in_=ew_d[:])
            et_sb = cpool.tile([S, W * BPC], mybir.dt.bfloat16)
            nc.sync.dma_start(out=et_sb[:], in_=et_d[:])

            for _ in range(repeats):
                if dma_only:
                    # bandwidth microbench: just stream all L slices, then one
                    # DVE touch per slice so the pool recycles realistically
                    for g in range(L):
                        es = espool.tile([S, N], mybir.dt.bfloat16, tag="es")
                        nc.sync.dma_start(out=es[:], in_=es_d[g])
                        nc.vector.tensor_copy(out=et_sb[:], in_=es[:, 0:W * BPC])
                    continue
                pers = []
                for g in range(W):
                    if g == 0 and pers0 is not None:
                        pers.append(pers0)
                        pers0 = None
                        continue
                    pt = perspool.tile([S, N], mybir.dt.bfloat16, tag="pers")
                    if not no_dma:
                        if g < psplit:
                            for q in range(N // gw):
                                nc.sync.dma_start(
                                    out=pt[:, q * gw:(q + 1) * gw],
                                    in_=es_d[g, :, q * gw:(q + 1) * gw],
                                )
                        else:
                            nc.sync.dma_start(out=pt[:], in_=es_d[g])
                    pers.append(pt)

                X = xpool.tile([S, N], mybir.dt.bfloat16, tag="X")
                nc.vector.memset(X[:], 1.0)

                hoisted = {}
                if hoist and not no_dma:
                    for g in range(W, L):
                        est = espool.tile([S, N], mybir.dt.bfloat16, tag="es")
                        nc.sync.dma_start(out=est[:], in_=es_d[g])
                        hoisted[g] = est

                for g in range(G):
                    if g < W:
                        es, off = pers[g], 0
                    elif g < L:
                        if no_dma:
                            es, off = pers[g % W], 0
                        elif hoist:
                            es, off = hoisted[g], 0
                        else:
                            es = espool.tile([S, N], mybir.dt.bfloat16, tag="es")
                            if esplit:
                                for q in range(ng):
                                    nc.sync.dma_start(
                                        out=es[:, q * gw:(q + 1) * gw],
                                        in_=es_d[g, :, q * gw:(q + 1) * gw],
                                    )
                            else:
                                nc.sync.dma_start(out=es[:], in_=es_d[g])
                            off = 0
                    else:
                        es, off = pers[g - L], BPC   # shifted warmup reuse

                    Xn = xpool.tile([S, N], mybir.dt.bfloat16, tag="X")
                    korder = (
                        list(range(ndirect, ng)) + list(range(ndirect))
                        if dlast else range(ng)
                    )
                    for k in korder:
                        lo, hi = k * gw, (k + 1) * gw
                        ps = ppool.tile([S, gw], mybir.dt.float32, tag=f"ps{k}")
                        nc.tensor.matmul(
                            out=ps[:], lhsT=E_sb[:], rhs=X[:, lo:hi],
                            start=True, stop=True,
                        )
                        # cols [lo:hi) multiply with es[:, lo+off : hi+off);
                        # the last group's tail (beyond N) comes from etail.
                        w = gw if hi + off <= N else gw - off
                        if k < ndirect:
                            nc.vector.tensor_mul(
                                out=Xn[:, lo:lo + w], in0=ps[:, 0:w],
                                in1=es[:, lo + off:lo + off + w],
                            )
                            if w < gw:
                                j = g - L
                                nc.vector.tensor_mul(
                                    out=Xn[:, lo + w:hi], in0=ps[:, w:gw],
                                    in1=et_sb[:, j * BPC:(j + 1) * BPC],
                                )
                        else:
                            ev = evpool.tile([S, gw], mybir.dt.bfloat16, tag=f"ev{k}")
                            nc.scalar.copy(out=ev[:], in_=ps[:])
                            nc.vector.tensor_mul(
                                out=Xn[:, lo:lo + w], in0=ev[:, 0:w],
                                in1=es[:, lo + off:lo + off + w],
                            )
                            if w < gw:
                                j = g - L
                                nc.vector.tensor_mul(
                                    out=Xn[:, lo + w:hi], in0=ev[:, w:gw],
                                    in1=et_sb[:, j * BPC:(j + 1) * BPC],
                                )
                    if g == W - 1:
                        # snapshot warmup state, then exact chunk-0 init:
                        # es_d[W-1] cols 0:BPC == exp(obs[:,0,:]-beta)
                        if park:
                            wpark = cpool.tile([S, N], mybir.dt.bfloat16)
                            nc.vector.tensor_copy(out=wpark[:], in_=Xn[:])
                        else:
                            for k in range(ng):
                                nc.sync.dma_start(
                                    out=wout_d[:, k * gw:(k + 1) * gw],
                                    in_=Xn[:, k * gw:(k + 1) * gw],
                                )
                        nc.vector.tensor_copy(
                            out=Xn[:, 0:BPC], in_=pers[W - 1][:, 0:BPC]
                        )
                    if park and wspread and W + 1 <= g < W + 1 + ng:
                        k = g - (W + 1)
                        nc.sync.dma_start(
                            out=wout_d[:, k * gw:(k + 1) * gw],
                            in_=wpark[:, k * gw:(k + 1) * gw],
                        )
                    if g == G - 1:
                        for k in range(ng):
                            nc.sync.dma_start(
                                out=yout_d[:, k * gw:(k + 1) * gw],
                                in_=Xn[:, k * gw:(k + 1) * gw],
                            )
                        if park and not wspread:
                            nc.scalar.dma_start(out=wout_d[:], in_=wpark[:])
                    X = Xn
    nc.compile()
    return nc


LAST_RESULTS = None
EXEC_NS = None  # filled by bench.py


def prep_in_maps(log_observation: np.ndarray, log_transition: np.ndarray):
    """Host-side prep: exp, transpose-to-slice-major, shard per core."""
    E = np.exp(log_transition.astype(np.float64)).astype(np.float32)
    ew_bf = E.astype(bf16)

    eobs = np.exp(log_observation.astype(np.float32) - BETA).astype(bf16)  # [B,T,S]

    in_maps = []
    for k in range(NCORES):
        blk = eobs[k * BPC:(k + 1) * BPC]            # [BPC, T, S]
        # [S, BPC, W-1 + T] with left pad W-1 (ones)
        eT = np.ones((S, BPC, W - 1 + T), dtype=bf16)
        eT[:, :, W - 1:] = blk.transpose(2, 0, 1)
        # chain (c,b) at slice g uses t = L*c + g - (W-1) -> padded index L*c + g
        st_s, st_b, st_t = eT.strides
        view = np.lib.stride_tricks.as_strided(
            eT, shape=(L, S, C, BPC), strides=(st_t, st_s, L * st_t, st_b)
        )
        es = np.ascontiguousarray(view).reshape(L, S, N)
        # last chunk's tail: step g=L+j uses t = T-W+1+j .. T (t=T -> pad 1.0)
        et = np.ones((S, W * BPC), dtype=bf16)
        for j in range(W - 1):
            et[:, j * BPC:(j + 1) * BPC] = blk[:, T - W + 1 + j, :].T
        in_maps.append({"eslices": es, "etail": et, "ew": ew_bf})
    return in_maps


def stitch_outputs(results) -> np.ndarray:
    """fp64 host stitch of per-core wout/yout -> [B] -logZ."""
    cnt = np.full(C, G, dtype=np.float64)
    cnt[0] = L + 1        # exact init consumed obs[0] + L official steps
    cnt[C - 1] = G - 1    # last chunk consumed one unbiased pad column
    cntw = float(W)
    out = np.empty(B, dtype=np.float64)
    for k in range(NCORES):
        y = results[k]["yout"].astype(np.float64).reshape(S, C, BPC)
        w = results[k]["wout"].astype(np.float64).reshape(S, C, BPC)
        Sy = y.sum(axis=0)            # [C, BPC]
        Sw = w.sum(axis=0)
        ly = np.log(Sy) + (BETA * cnt)[:, None]
        lw = np.log(Sw) + BETA * cntw
        logZ = ly[C - 1] + np.sum(ly[: C - 1] - lw[1:], axis=0)
        out[k * BPC:(k + 1) * BPC] = -logZ
    return out


def kernel(log_observation: np.ndarray, log_transition: np.ndarray) -> np.ndarray:
    assert log_observation.shape == (B, T, S)
    assert log_transition.shape == (S, S)

    in_maps = prep_in_maps(log_observation, log_transition)
    nc = _build_device_program()
    res = run_bass_kernel_spmd(nc, in_maps, core_ids=list(range(NCORES)))
    global LAST_RESULTS
    LAST_RESULTS = res
    return stitch_outputs(res.results).astype(np.float32)

